# revision 38
# baseline (speedup 1.0000x reference)
"""Multi-head attention (B=2, S=2048, D=1024, H=16) on 8 NeuronCores.

Sharding: core c -> batch c//4, head-group c%4 (4 heads, 256 proj dims).
Per-core Bass/Tile kernel: bf16 Q/K/V projections (weight-stationary),
transposed-scores attention in 512-wide q-chunks (k on partitions,
softmax without max-subtraction via a ones-column rowsum), row-parallel
partial out-projection. Host sums the 4 bf16 partials per batch + bo.

Perf notes (HW-measured):
- All matmul operands bf16 (f32r moving data doubles SBUF read power and
  feeds the chip's activity throttle; fp8 DoubleRow would halve PE rows
  but its ~5% rms error fails the 2e-2 gate).
- TRN2 duty-cycle controller: sustained PE+ACT concurrency escalates to
  a 50% utilization clamp after ~45us; low-activity phases (projections,
  out-proj) run unthrottled. 512-wide chunks + out-proj interleaved at
  the second head-pair keep activity dips frequent so the controller
  keeps de-escalating.
- Weights/biases host-packed so every SBUF partition row is one
  contiguous DRAM line (one DMA per tensor, 4KB packets instead of
  128x512B).
- Out-proj PSUM tiles staged to SBUF via ACT-engine Copy (DVE is busier;
  GpSimd cannot read PSUM); y shipped bf16.
- exp on ACT is the irreducible floor (~131k lanes-cycles/core); scores
  psum double-buffered (bufs=2) so scores run ahead of exp.
- Non-critical const DMAs (wq/wk/wo/biases/ident) are dribbled one per
  x-chunk behind the first V-proj loads: the Sync queue issues in order
  at ~0.7us per DMA, so front-loading them delays the first matmul.
- DVE reciprocal() is a ~3.3us multipass op per [1,512] call; the
  tail-critical last-chunk normalize broadcasts 1/rowsum via a one-row
  PE matmul instead of gpsimd (PE is idle there). reciprocal_approx_*
  custom-DVE ops produce garbage on this hardware; ACT Reciprocal is
  hard-blocked by bass.
- Both head-halves' scores share one [128,1024] psum tile so each kt
  step needs ONE exp instead of two: halving ACT instruction/semaphore
  rate dropped the chip's activity escalation below threshold
  (throttle_active 110us -> 28us) and made the attention phase purely
  ACT-exp-bound at ~1.04us/kt.
- Q-projection chunks 1-3 are deferred into the attention phase as
  pair-boundary PE filler (their x-chunks stay in the xin rotation),
  starting the exp stream ~14us earlier. The same trick for K regressed
  (mid-kt-loop filler bubbles the saturated ACT stream) - pair
  boundaries are the only safe filler slots.
"""

import sys

sys.path.insert(0, "/opt/trn_rl_repo")

from contextlib import ExitStack

import numpy as np

import concourse.bacc as bacc
import concourse.mybir as mybir
import concourse.tile as tile
from concourse.bass_utils import run_bass_kernel_spmd

B = 2
S = 2048
D = 1024
H = 16
HD = 64
HPC = 4          # heads per core
DPC = HPC * HD   # 256 projection dims per core
NCORES = 8
SCALE = 8.0      # sqrt(HD)

F32 = mybir.dt.float32
F32R = mybir.dt.float32r
BF16 = mybir.dt.bfloat16
ADT = BF16   # attention operand dtype (qt/kt/v/pt)

DCH = D // 128   # 8 contraction chunks of 128
QT = S // 128    # 16 q-tiles / k-tiles of 128
QCN = 4          # attention q-chunks of 512
QCW = 512


def build_nc():
    nc = bacc.Bacc("TRN2", target_bir_lowering=False, debug=False, num_devices=NCORES)

    xq = nc.dram_tensor("xq_t", [D, S], BF16, kind="ExternalInput")
    xk = nc.dram_tensor("xk_t", [D, S], BF16, kind="ExternalInput")
    xv = nc.dram_tensor("xv_t", [D, S], BF16, kind="ExternalInput")
    wq = nc.dram_tensor("wq_t", [128, D // 128 * DPC], BF16, kind="ExternalInput")
    wk = nc.dram_tensor("wk_t", [128, D // 128 * DPC], BF16, kind="ExternalInput")
    wv = nc.dram_tensor("wv_t", [128, D // 128 * DPC], BF16, kind="ExternalInput")
    wo = nc.dram_tensor("wo_t", [DPC, D], BF16, kind="ExternalInput")
    ball = nc.dram_tensor("ball", [128, 6], F32, kind="ExternalInput")
    ident = nc.dram_tensor("ident", [128, 128], BF16, kind="ExternalInput")
    y = nc.dram_tensor("y", [S, D], BF16, kind="ExternalOutput")

    with tile.TileContext(nc) as tc, ExitStack() as ctx:
        const = ctx.enter_context(tc.tile_pool(name="const", bufs=1))
        xin = ctx.enter_context(tc.tile_pool(name="xin", bufs=8))
        qkv = ctx.enter_context(tc.tile_pool(name="qkv", bufs=1))
        yp = ctx.enter_context(tc.tile_pool(name="yp", bufs=3))
        ptp = ctx.enter_context(tc.tile_pool(name="ptp", bufs=3))
        nrm = ctx.enter_context(tc.tile_pool(name="nrm", bufs=2))
        # tag semantics: same tag -> rotate through `bufs` slots;
        # distinct tags -> independent allocations.

        # ---- constants / weights ----
        # tiny dummy exp first: preloads the ACT exp table set off the
        # critical path (a ~4us PE-idle gap at attention start re-throttles
        # the PE clock to 1.2GHz for the rest of the kernel otherwise)
        dmy = const.tile([1, 16], F32, tag="dmy")
        nc.vector.memset(dmy[:], 0.0)
        dmy2 = const.tile([1, 16], F32, tag="dmy2")
        nc.scalar.activation(dmy2[:], dmy[:], mybir.ActivationFunctionType.Exp)

        # memset can't target f32r; stage in f32 and round via DVE copy
        ones32 = const.tile([1, 128], F32, tag="ones32")
        nc.vector.memset(ones32[:], 1.0)
        ones = const.tile([1, 128], F32R, tag="ones")
        nc.vector.tensor_copy(ones[:], ones32[:])
        ones_r32 = const.tile([1, HD], F32, tag="ones_r32")
        nc.vector.memset(ones_r32[:], 1.0)
        ones_row = const.tile([1, HD], BF16, tag="ones_row")
        nc.vector.tensor_copy(ones_row[:], ones_r32[:])
        onesv32 = const.tile([128, HPC], F32, tag="onesv32")
        nc.vector.memset(onesv32[:], 1.0)
        onesv = const.tile([128, HPC], F32R, tag="onesv")
        nc.vector.tensor_copy(onesv[:], onesv32[:])
        # packed weights: one contiguous-row DMA per tensor (vs 8x128
        # 512B-packet transfers); w*_sb[d] are views into the packed tile
        wq_all = const.tile([128, DCH * DPC], BF16, tag="wqa", name="wqa")
        wk_all = const.tile([128, DCH * DPC], BF16, tag="wka", name="wka")
        wv_all = const.tile([128, DCH * DPC], BF16, tag="wva", name="wva")
        nc.sync.dma_start(wv_all[:, 0:DPC], wv[:, 0:DPC])
        nc.sync.dma_start(wv_all[:, DPC:], wv[:, DPC:])
        ball_sb = const.tile([128, 6], F32, tag="ball", name="ball")
        # deferred until after the first x-chunk DMA is issued: only wv and
        # xt(v,0) gate the first matmul, and the Sync queue issues in order
        deferred_dmas = [
            lambda: nc.scalar.dma_start(wq_all[:], wq[:]),
            lambda: nc.scalar.dma_start(wk_all[:], wk[:]),
            lambda: nc.scalar.dma_start(ball_sb[:], ball[:]),
        ]
        wq_sb = [wq_all[:, d * DPC:(d + 1) * DPC] for d in range(DCH)]
        wk_sb = [wk_all[:, d * DPC:(d + 1) * DPC] for d in range(DCH)]
        wv_sb = [wv_all[:, d * DPC:(d + 1) * DPC] for d in range(DCH)]
        wo_sb = [const.tile([128, D], BF16, tag=f"wo{g}", name=f"wo{g}") for g in range(2)]
        for g in range(2):
            deferred_dmas.append(
                lambda g=g: nc.scalar.dma_start(wo_sb[g][:], wo[g * 128:(g + 1) * 128, :]))
        bq_sb = [ball_sb[:, hp:hp + 1] for hp in range(2)]
        bk_sb = [ball_sb[:, 2 + hp:3 + hp] for hp in range(2)]
        bv_sb = [ball_sb[:, 4 + hp:5 + hp] for hp in range(2)]
        id_sb = const.tile([128, 128], BF16, tag="id")
        deferred_dmas.append(lambda: nc.scalar.dma_start(id_sb[:], ident[:]))

        # ---- V tiles (128, 4*65) with ones column, filled by PE transpose
        # of a V.T projection (weight-stationary like Q/K; avoids the
        # per-matmul LDWEIGHTS serialization of an x-stationary V-proj) ----
        v_sb = [qkv.tile([128, HPC * (HD + 1)], ADT, tag=f"v{st}", name=f"v{st}") for st in range(QT)]
        for st in range(QT):
            v4 = v_sb[st][:].rearrange("p (h w) -> p h w", h=HPC)
            nc.vector.tensor_copy(
                v4[:, :, HD:HD + 1],
                onesv[:].rearrange("p (a b) -> p a b", b=1),
            )
        vt_sb = [qkv.tile([128, S], BF16, tag=f"vt{hp}", name=f"vtt{hp}") for hp in range(2)]

        # ---- Q.T / K.T projections: (d'=hp*128 partitions, s free) ----
        qt_sb = [qkv.tile([128, S], ADT, tag=f"qt{hp}", name=f"qtt{hp}") for hp in range(2)]
        kt_sb = [qkv.tile([128, S], ADT, tag=f"kt{hp}", name=f"ktt{hp}") for hp in range(2)]
        with tc.tile_pool(name="ps_p", bufs=1, space="PSUM") as ps_p:
            xq_tiles = []
            for which, xin_dram, w_sb, b_sb, dst in (
                ("v", xv, wv_sb, bv_sb, vt_sb),
                ("k", xk, wk_sb, bk_sb, kt_sb),
                ("q", xq, wq_sb, bq_sb, qt_sb),
            ):
                pcs = [0] if which == "q" else [0, 1, 2, 3]
                accs = {}
                for hp in range(2):
                    for pc in pcs:
                        accs[(hp, pc)] = ps_p.tile([128, 512], F32, tag=f"pp{hp * 4 + pc}", name=f"pp_{which}{hp}{pc}")
                for d in range(DCH):
                    xt = xin.tile([128, S], BF16, tag="x")
                    if which == "v" and d == 0:
                        nc.sync.dma_start(xt[:, 0:512],
                                          xin_dram[0:128, 0:512])
                        nc.sync.dma_start(xt[:, 512:],
                                          xin_dram[0:128, 512:])
                    elif which == "q":
                        # only the pc0 slice is needed before attention; the
                        # rest streams in during attention (DMA is idle then)
                        nc.sync.dma_start(xt[:, 0:512],
                                          xin_dram[d * 128:(d + 1) * 128, 0:512])
                    else:
                        # alternate issue queues: the Sync engine's ~0.7us
                        # per-DMA issue time is a pacing floor for the
                        # DMA-heavy projection phase; ACT is idle here
                        eng = nc.sync if d % 2 == 0 else nc.scalar
                        eng.dma_start(xt[:], xin_dram[d * 128:(d + 1) * 128, :])
                    if which == "q":
                        xq_tiles.append(xt)
                    if deferred_dmas:
                        deferred_dmas.pop(0)()
                    for hp in range(2):
                        for pc in pcs:
                            nc.tensor.matmul(
                                accs[(hp, pc)][:],
                                w_sb[d][:, hp * 128:(hp + 1) * 128],
                                xt[:, pc * 512:(pc + 1) * 512],
                                start=(d == 0), stop=(d == DCH - 1),
                            )
                for hp in range(2):
                    for pc in pcs:
                        nc.vector.tensor_scalar_add(
                            dst[hp][:, pc * 512:(pc + 1) * 512],
                            accs[(hp, pc)][:],
                            b_sb[hp][:],
                        )
            # V.T -> V transposes last: dense PE work (~8us) bridging the
            # proj->attention boundary while K/Q evacuations drain, so the
            # PE clock stays un-throttled into the attention phase
            for hp in range(2):
                for st in range(QT):
                    tp = ps_p.tile([128, 128], BF16, tag=f"pp{st % 8}",
                                   name=f"tp{hp}{st}")
                    nc.tensor.transpose(
                        tp[:],
                        vt_sb[hp][:, st * 128:(st + 1) * 128],
                        id_sb[:],
                    )
                    v4 = v_sb[st][:].rearrange("p (h w) -> p h w", h=HPC)
                    nc.vector.tensor_copy(
                        v4[:, 2 * hp:2 * hp + 2, 0:HD],
                        tp[:].rearrange("p (h w) -> p h w", h=2),
                    )

        # ---- attention + normalization, head-pairs packed on PE rows ----
        otn_sb = [qkv.tile([128, S], BF16, tag=f"otn{j}", name=f"otn{j}") for j in range(2)]
        with tc.tile_pool(name="ps_s", bufs=2, space="PSUM") as ps_s, \
             tc.tile_pool(name="ps_o", bufs=1, space="PSUM") as ps_o, \
             tc.tile_pool(name="ps_q", bufs=1, space="PSUM") as ps_q:

            def emit_qproj(pc):
                # deferred Q-projection chunk: PE filler at a pair boundary,
                # so the exp stream starts ~3 chunks of q-proj earlier
                for hp in range(2):
                    acc = ps_q.tile([128, 512], F32, tag=f"qp{hp}",
                                    name=f"qp{pc}{hp}")
                    for d in range(DCH):
                        nc.tensor.matmul(
                            acc[:],
                            wq_sb[d][:, hp * 128:(hp + 1) * 128],
                            xq_tiles[d][:, pc * 512:(pc + 1) * 512],
                            start=(d == 0), stop=(d == DCH - 1),
                        )
                    nc.vector.tensor_scalar_add(
                        qt_sb[hp][:, pc * 512:(pc + 1) * 512],
                        acc[:], bq_sb[hp][:],
                    )

            def emit_outproj(qc):
                # out-proj for a finished q-chunk; emitted during the NEXT
                # chunk's attention so its matmuls fill PE slack there
                for qt_i in range(qc * QCW // 128, (qc + 1) * QCW // 128):
                    ysb = yp.tile([128, D], BF16, tag="y", name=f"ysb{qt_i}")
                    for dc in range(2):
                        yps = ps_o.tile([128, 512], F32, tag=f"ot{dc}",
                                        name=f"yps{qt_i}{dc}")
                        for g in range(2):
                            nc.tensor.matmul(
                                yps[:],
                                otn_sb[g][:, qt_i * 128:(qt_i + 1) * 128],
                                wo_sb[g][:, dc * 512:(dc + 1) * 512],
                                start=(g == 0), stop=(g == 1),
                            )
                        nc.scalar.activation(
                            ysb[:, dc * 512:(dc + 1) * 512], yps[:],
                            mybir.ActivationFunctionType.Copy,
                        )
                    nc.sync.dma_start(y[qt_i * 128:(qt_i + 1) * 128, :], ysb[:])

            for d in range(DCH):
                nc.sync.dma_start(xq_tiles[d][:, 512:S],
                                  xq[d * 128:(d + 1) * 128, 512:S])

            pending = []
            for qc in range(QCN):
                for j in range(2):          # head pair: heads 2j, 2j+1
                    if j == 1 and qc < 3:
                        emit_qproj(qc + 1)
                    if j == 1 and pending:
                        emit_outproj(pending.pop())
                    ot_ps = [ps_o.tile([HD + 1, QCW], F32, tag=f"ot{h2}", name=f"ot{qc}{j}{h2}")
                             for h2 in range(2)]
                    pts = {}
                    for kt in range(QT):
                        # both head-halves' scores in ONE psum tile -> ONE
                        # [128,1024] exp per kt (halves ACT instruction and
                        # semaphore overhead; attention is ACT-bound at full
                        # PE speed)
                        sps = ps_s.tile([128, 2 * QCW], F32, tag="s",
                                        name=f"sps{kt % 2}")
                        for h2 in range(2):  # h2=0 -> rows 0:64, h2=1 -> 64:128
                            nc.tensor.matmul(
                                sps[:, h2 * QCW:(h2 + 1) * QCW],
                                kt_sb[j][h2 * 64:h2 * 64 + 64,
                                         kt * 128:(kt + 1) * 128],
                                qt_sb[j][h2 * 64:h2 * 64 + 64,
                                         qc * QCW:(qc + 1) * QCW],
                                start=True, stop=True,
                                tile_position=(h2 * 64, 0),
                            )
                        pt = ptp.tile([128, 2 * QCW], ADT, tag="pt")
                        nc.scalar.activation(
                            pt[:], sps[:],
                            mybir.ActivationFunctionType.Exp,
                            scale=1.0 / SCALE,
                        )
                        for h2 in range(2):
                            h = 2 * j + h2
                            nc.tensor.matmul(
                                ot_ps[h2][:],
                                v_sb[kt][:, h * 65:h * 65 + 65],
                                pt[:, h2 * QCW:(h2 + 1) * QCW],
                                start=(kt == 0), stop=(kt == QT - 1),
                            )
                    # evacuate O.T+sums to SBUF fast (frees the psum slot
                    # for the next head pair), then normalize off-path
                    for h2 in range(2):
                        h = 2 * j + h2
                        otr = nrm.tile([HD + 1, QCW], BF16, tag="otr")
                        nc.vector.tensor_copy(otr[:], ot_ps[h2][:])
                        rc32 = nrm.tile([1, QCW], BF16, tag="rc32")
                        with nc.allow_low_precision(reason="bf16 softmax denom; tol 2e-2"):
                            nc.vector.reciprocal(rc32[:], otr[HD:HD + 1, :])
                        if qc == QCN - 1 and j == 1:
                            # tail-critical normalize: broadcast 1/rowsum via a
                            # one-row PE matmul (PE is idle here; gpsimd's
                            # software broadcast would sit on the outproj path)
                            sc_ps = ps_s.tile([128, 2 * QCW], F32, tag="s",
                                              name=f"scps{h2}")
                            nc.tensor.matmul(sc_ps[0:HD, 0:QCW], ones_row[:],
                                             rc32[:], start=True, stop=True)
                            nc.vector.tensor_mul(
                                otn_sb[j][h2 * HD:(h2 + 1) * HD,
                                          qc * QCW:(qc + 1) * QCW],
                                otr[0:HD, :], sc_ps[0:HD, 0:QCW],
                            )
                        else:
                            sc = nrm.tile([HD, QCW], BF16, tag="sc")
                            nc.gpsimd.partition_broadcast(sc[:], rc32[:])
                            nc.vector.tensor_mul(
                                otn_sb[j][h2 * HD:(h2 + 1) * HD,
                                          qc * QCW:(qc + 1) * QCW],
                                otr[0:HD, :], sc[:],
                            )
                pending.append(qc)
            emit_outproj(pending.pop())

    nc.compile()
    return nc


_NC_CACHE = None


def _get_nc():
    global _NC_CACHE
    if _NC_CACHE is None:
        _NC_CACHE = build_nc()
    return _NC_CACHE


def _pack_w(Whs):
    """W[256, 1024] -> W.T [1024, 256] -> [128, 8*256] with row p holding
    contraction rows {p, 128+p, ...}: one contiguous 4KB DMA line per
    partition."""
    import ml_dtypes
    wt = np.ascontiguousarray(np.asarray(Whs, np.float32).T)  # [1024, 256]
    packed = wt.reshape(8, 128, 256).transpose(1, 0, 2).reshape(128, 8 * 256)
    return np.ascontiguousarray(packed).astype(ml_dtypes.bfloat16)


def shard_inputs(query, key, value, Wq, bq, Wk, bk, Wv, bv, Wo, bo):
    """Build the 8 per-core input maps (host-side shard + transpose)."""
    import ml_dtypes
    f = np.float32
    bf = ml_dtypes.bfloat16
    in_maps = []
    for c in range(NCORES):
        b = c // 4
        g = c % 4
        hs = slice(g * DPC, (g + 1) * DPC)
        in_maps.append({
            "xq_t": np.ascontiguousarray(np.asarray(query[b], f).T).astype(bf),
            "xk_t": np.ascontiguousarray(np.asarray(key[b], f).T).astype(bf),
            "xv_t": np.ascontiguousarray(np.asarray(value[b], f).T).astype(bf),
            "wq_t": _pack_w(Wq[hs, :]),
            "wk_t": _pack_w(Wk[hs, :]),
            "wv_t": _pack_w(Wv[hs, :]),
            "wo_t": np.ascontiguousarray(np.asarray(Wo[:, hs], f).T).astype(bf),
            "ball": np.stack([np.asarray(b[hs], f).reshape(2, 128)[hp]
                              for b in (bq, bk, bv) for hp in range(2)],
                             axis=1).copy(),
            "ident": np.eye(128, dtype=f).astype(bf),
        })
    return in_maps


def kernel(query, key, value, Wq, bq, Wk, bk, Wv, bv, Wo, bo, **run_kwargs):
    nc = _get_nc()
    in_maps = shard_inputs(query, key, value, Wq, bq, Wk, bk, Wv, bv, Wo, bo)
    res = run_bass_kernel_spmd(nc, in_maps, core_ids=list(range(NCORES)),
                               **run_kwargs)
    out = np.zeros((B, S, D), np.float32)
    for c in range(NCORES):
        out[c // 4] += np.asarray(res.results[c]["y"], np.float32)
    out += np.asarray(bo, np.float32)
    if run_kwargs:
        kernel.last_result = res
    return out



# revision 39
# speedup vs baseline: 1.0089x; 1.0089x over previous
"""Multi-head attention (B=2, S=2048, D=1024, H=16) on 8 NeuronCores.

Sharding: core c -> batch c//4, head-group c%4 (4 heads, 256 proj dims).
Per-core Bass/Tile kernel: bf16 Q/K/V projections (weight-stationary),
transposed-scores attention in 512-wide q-chunks (k on partitions,
softmax without max-subtraction via a ones-column rowsum), row-parallel
partial out-projection. Host sums the 4 bf16 partials per batch + bo.

Perf notes (HW-measured):
- All matmul operands bf16 (f32r moving data doubles SBUF read power and
  feeds the chip's activity throttle; fp8 DoubleRow would halve PE rows
  but its ~5% rms error fails the 2e-2 gate).
- TRN2 duty-cycle controller: sustained PE+ACT concurrency escalates to
  a 50% utilization clamp after ~45us; low-activity phases (projections,
  out-proj) run unthrottled. 512-wide chunks + out-proj interleaved at
  the second head-pair keep activity dips frequent so the controller
  keeps de-escalating.
- Weights/biases host-packed so every SBUF partition row is one
  contiguous DRAM line (one DMA per tensor, 4KB packets instead of
  128x512B).
- Out-proj PSUM tiles staged to SBUF via ACT-engine Copy (DVE is busier;
  GpSimd cannot read PSUM); y shipped bf16.
- exp on ACT is the irreducible floor (~131k lanes-cycles/core); scores
  psum double-buffered (bufs=2) so scores run ahead of exp.
- Non-critical const DMAs (wq/wk/wo/biases/ident) are dribbled one per
  x-chunk behind the first V-proj loads: the Sync queue issues in order
  at ~0.7us per DMA, so front-loading them delays the first matmul.
- DVE reciprocal() is a ~3.3us multipass op per [1,512] call; the
  tail-critical last-chunk normalize broadcasts 1/rowsum via a one-row
  PE matmul instead of gpsimd (PE is idle there). reciprocal_approx_*
  custom-DVE ops produce garbage on this hardware; ACT Reciprocal is
  hard-blocked by bass.
- Both head-halves' scores share one [128,1024] psum tile so each kt
  step needs ONE exp instead of two: halving ACT instruction/semaphore
  rate dropped the chip's activity escalation below threshold
  (throttle_active 110us -> 28us) and made the attention phase purely
  ACT-exp-bound at ~1.04us/kt.
- Q-projection chunks 1-3 are deferred into the attention phase as
  pair-boundary PE filler (their x-chunks stay in the xin rotation),
  starting the exp stream ~14us earlier. The same trick for K regressed
  (mid-kt-loop filler bubbles the saturated ACT stream) - pair
  boundaries are the only safe filler slots.
"""

import sys

sys.path.insert(0, "/opt/trn_rl_repo")

from contextlib import ExitStack

import numpy as np

import concourse.bacc as bacc
import concourse.mybir as mybir
import concourse.tile as tile
from concourse.bass_utils import run_bass_kernel_spmd

B = 2
S = 2048
D = 1024
H = 16
HD = 64
HPC = 4          # heads per core
DPC = HPC * HD   # 256 projection dims per core
NCORES = 8
SCALE = 8.0      # sqrt(HD)

F32 = mybir.dt.float32
F32R = mybir.dt.float32r
BF16 = mybir.dt.bfloat16
ADT = BF16   # attention operand dtype (qt/kt/v/pt)

DCH = D // 128   # 8 contraction chunks of 128
QT = S // 128    # 16 q-tiles / k-tiles of 128
QCN = 4          # attention q-chunks of 512
QCW = 512


def build_nc():
    nc = bacc.Bacc("TRN2", target_bir_lowering=False, debug=False, num_devices=NCORES)

    xq = nc.dram_tensor("xq_t", [D, S], BF16, kind="ExternalInput")
    xk = nc.dram_tensor("xk_t", [D, S], BF16, kind="ExternalInput")
    xv = nc.dram_tensor("xv_t", [D, S], BF16, kind="ExternalInput")
    wq = nc.dram_tensor("wq_t", [128, D // 128 * DPC], BF16, kind="ExternalInput")
    wk = nc.dram_tensor("wk_t", [128, D // 128 * DPC], BF16, kind="ExternalInput")
    wv = nc.dram_tensor("wv_t", [128, D // 128 * DPC], BF16, kind="ExternalInput")
    wo = nc.dram_tensor("wo_t", [DPC, D], BF16, kind="ExternalInput")
    ball = nc.dram_tensor("ball", [128, 6], F32, kind="ExternalInput")
    ident = nc.dram_tensor("ident", [128, 128], BF16, kind="ExternalInput")
    y = nc.dram_tensor("y", [S, D], BF16, kind="ExternalOutput")

    with tile.TileContext(nc) as tc, ExitStack() as ctx:
        const = ctx.enter_context(tc.tile_pool(name="const", bufs=1))
        xin = ctx.enter_context(tc.tile_pool(name="xin", bufs=8))
        qkv = ctx.enter_context(tc.tile_pool(name="qkv", bufs=1))
        yp = ctx.enter_context(tc.tile_pool(name="yp", bufs=3))
        ptp = ctx.enter_context(tc.tile_pool(name="ptp", bufs=3))
        nrm = ctx.enter_context(tc.tile_pool(name="nrm", bufs=2))
        # tag semantics: same tag -> rotate through `bufs` slots;
        # distinct tags -> independent allocations.

        # ---- constants / weights ----
        # tiny dummy exp first: preloads the ACT exp table set off the
        # critical path (a ~4us PE-idle gap at attention start re-throttles
        # the PE clock to 1.2GHz for the rest of the kernel otherwise)
        dmy = const.tile([1, 16], F32, tag="dmy")
        nc.vector.memset(dmy[:], 0.0)
        dmy2 = const.tile([1, 16], F32, tag="dmy2")
        nc.scalar.activation(dmy2[:], dmy[:], mybir.ActivationFunctionType.Exp)

        # memset can't target f32r; stage in f32 and round via DVE copy
        ones32 = const.tile([1, 128], F32, tag="ones32")
        nc.vector.memset(ones32[:], 1.0)
        ones = const.tile([1, 128], F32R, tag="ones")
        nc.vector.tensor_copy(ones[:], ones32[:])
        ones_r32 = const.tile([1, HD], F32, tag="ones_r32")
        nc.vector.memset(ones_r32[:], 1.0)
        ones_row = const.tile([1, HD], BF16, tag="ones_row")
        nc.vector.tensor_copy(ones_row[:], ones_r32[:])
        onesv32 = const.tile([128, HPC], F32, tag="onesv32")
        nc.vector.memset(onesv32[:], 1.0)
        onesv = const.tile([128, HPC], F32R, tag="onesv")
        nc.vector.tensor_copy(onesv[:], onesv32[:])
        # packed weights: one contiguous-row DMA per tensor (vs 8x128
        # 512B-packet transfers); w*_sb[d] are views into the packed tile
        wq_all = const.tile([128, DCH * DPC], BF16, tag="wqa", name="wqa")
        wk_all = const.tile([128, DCH * DPC], BF16, tag="wka", name="wka")
        wv_all = const.tile([128, DCH * DPC], BF16, tag="wva", name="wva")
        nc.sync.dma_start(wv_all[:, 0:DPC], wv[:, 0:DPC])
        nc.sync.dma_start(wv_all[:, DPC:], wv[:, DPC:])
        ball_sb = const.tile([128, 6], F32, tag="ball", name="ball")
        # deferred until after the first x-chunk DMA is issued: only wv and
        # xt(v,0) gate the first matmul, and the Sync queue issues in order
        deferred_dmas = [
            lambda: nc.sync.dma_start(wq_all[:], wq[:]),
            lambda: nc.sync.dma_start(wk_all[:], wk[:]),
            lambda: nc.sync.dma_start(ball_sb[:], ball[:]),
        ]
        wq_sb = [wq_all[:, d * DPC:(d + 1) * DPC] for d in range(DCH)]
        wk_sb = [wk_all[:, d * DPC:(d + 1) * DPC] for d in range(DCH)]
        wv_sb = [wv_all[:, d * DPC:(d + 1) * DPC] for d in range(DCH)]
        wo_sb = [const.tile([128, D], BF16, tag=f"wo{g}", name=f"wo{g}") for g in range(2)]
        for g in range(2):
            deferred_dmas.append(
                lambda g=g: nc.sync.dma_start(wo_sb[g][:], wo[g * 128:(g + 1) * 128, :]))
        bq_sb = [ball_sb[:, hp:hp + 1] for hp in range(2)]
        bk_sb = [ball_sb[:, 2 + hp:3 + hp] for hp in range(2)]
        bv_sb = [ball_sb[:, 4 + hp:5 + hp] for hp in range(2)]
        id_sb = const.tile([128, 128], BF16, tag="id")
        deferred_dmas.append(lambda: nc.sync.dma_start(id_sb[:], ident[:]))

        # ---- V tiles (128, 4*65) with ones column, filled by PE transpose
        # of a V.T projection (weight-stationary like Q/K; avoids the
        # per-matmul LDWEIGHTS serialization of an x-stationary V-proj) ----
        v_sb = [qkv.tile([128, HPC * (HD + 1)], ADT, tag=f"v{st}", name=f"v{st}") for st in range(QT)]
        for st in range(QT):
            v4 = v_sb[st][:].rearrange("p (h w) -> p h w", h=HPC)
            nc.vector.tensor_copy(
                v4[:, :, HD:HD + 1],
                onesv[:].rearrange("p (a b) -> p a b", b=1),
            )
        vt_sb = [qkv.tile([128, S], BF16, tag=f"vt{hp}", name=f"vtt{hp}") for hp in range(2)]

        # ---- Q.T / K.T projections: (d'=hp*128 partitions, s free) ----
        qt_sb = [qkv.tile([128, S], ADT, tag=f"qt{hp}", name=f"qtt{hp}") for hp in range(2)]
        kt_sb = [qkv.tile([128, S], ADT, tag=f"kt{hp}", name=f"ktt{hp}") for hp in range(2)]
        with tc.tile_pool(name="ps_p", bufs=1, space="PSUM") as ps_p:
            xq_tiles = []
            for which, xin_dram, w_sb, b_sb, dst in (
                ("v", xv, wv_sb, bv_sb, vt_sb),
                ("k", xk, wk_sb, bk_sb, kt_sb),
                ("q", xq, wq_sb, bq_sb, qt_sb),
            ):
                pcs = [0] if which == "q" else [0, 1, 2, 3]
                accs = {}
                for hp in range(2):
                    for pc in pcs:
                        accs[(hp, pc)] = ps_p.tile([128, 512], F32, tag=f"pp{hp * 4 + pc}", name=f"pp_{which}{hp}{pc}")
                for d in range(DCH):
                    xt = xin.tile([128, S], BF16, tag="x")
                    if which == "v" and d == 0:
                        nc.sync.dma_start(xt[:, 0:512],
                                          xin_dram[0:128, 0:512])
                        nc.sync.dma_start(xt[:, 512:],
                                          xin_dram[0:128, 512:])
                    else:
                        nc.sync.dma_start(xt[:], xin_dram[d * 128:(d + 1) * 128, :])
                    if which == "q":
                        xq_tiles.append(xt)
                    if deferred_dmas:
                        deferred_dmas.pop(0)()
                    for hp in range(2):
                        for pc in pcs:
                            nc.tensor.matmul(
                                accs[(hp, pc)][:],
                                w_sb[d][:, hp * 128:(hp + 1) * 128],
                                xt[:, pc * 512:(pc + 1) * 512],
                                start=(d == 0), stop=(d == DCH - 1),
                            )
                for hp in range(2):
                    for pc in pcs:
                        nc.vector.tensor_scalar_add(
                            dst[hp][:, pc * 512:(pc + 1) * 512],
                            accs[(hp, pc)][:],
                            b_sb[hp][:],
                        )
            # V.T -> V transposes last: dense PE work (~8us) bridging the
            # proj->attention boundary while K/Q evacuations drain, so the
            # PE clock stays un-throttled into the attention phase
            for hp in range(2):
                for st in range(QT):
                    tp = ps_p.tile([128, 128], BF16, tag=f"pp{st % 8}",
                                   name=f"tp{hp}{st}")
                    nc.tensor.transpose(
                        tp[:],
                        vt_sb[hp][:, st * 128:(st + 1) * 128],
                        id_sb[:],
                    )
                    v4 = v_sb[st][:].rearrange("p (h w) -> p h w", h=HPC)
                    nc.vector.tensor_copy(
                        v4[:, 2 * hp:2 * hp + 2, 0:HD],
                        tp[:].rearrange("p (h w) -> p h w", h=2),
                    )

        # ---- attention + normalization, head-pairs packed on PE rows ----
        otn_sb = [qkv.tile([128, S], BF16, tag=f"otn{j}", name=f"otn{j}") for j in range(2)]
        with tc.tile_pool(name="ps_s", bufs=2, space="PSUM") as ps_s, \
             tc.tile_pool(name="ps_o", bufs=1, space="PSUM") as ps_o, \
             tc.tile_pool(name="ps_q", bufs=1, space="PSUM") as ps_q:

            def emit_qproj(pc):
                # deferred Q-projection chunk: PE filler at a pair boundary,
                # so the exp stream starts ~3 chunks of q-proj earlier
                for hp in range(2):
                    acc = ps_q.tile([128, 512], F32, tag=f"qp{hp}",
                                    name=f"qp{pc}{hp}")
                    for d in range(DCH):
                        nc.tensor.matmul(
                            acc[:],
                            wq_sb[d][:, hp * 128:(hp + 1) * 128],
                            xq_tiles[d][:, pc * 512:(pc + 1) * 512],
                            start=(d == 0), stop=(d == DCH - 1),
                        )
                    nc.vector.tensor_scalar_add(
                        qt_sb[hp][:, pc * 512:(pc + 1) * 512],
                        acc[:], bq_sb[hp][:],
                    )

            def emit_outproj(qc):
                # out-proj for a finished q-chunk; emitted during the NEXT
                # chunk's attention so its matmuls fill PE slack there
                for qt_i in range(qc * QCW // 128, (qc + 1) * QCW // 128):
                    ysb = yp.tile([128, D], BF16, tag="y", name=f"ysb{qt_i}")
                    for dc in range(2):
                        yps = ps_o.tile([128, 512], F32, tag=f"ot{dc}",
                                        name=f"yps{qt_i}{dc}")
                        for g in range(2):
                            nc.tensor.matmul(
                                yps[:],
                                otn_sb[g][:, qt_i * 128:(qt_i + 1) * 128],
                                wo_sb[g][:, dc * 512:(dc + 1) * 512],
                                start=(g == 0), stop=(g == 1),
                            )
                        nc.scalar.activation(
                            ysb[:, dc * 512:(dc + 1) * 512], yps[:],
                            mybir.ActivationFunctionType.Copy,
                        )
                    nc.sync.dma_start(y[qt_i * 128:(qt_i + 1) * 128, :], ysb[:])

            pending = []
            for qc in range(QCN):
                for j in range(2):          # head pair: heads 2j, 2j+1
                    if j == 1 and qc < 3:
                        emit_qproj(qc + 1)
                    if j == 1 and pending:
                        emit_outproj(pending.pop())
                    ot_ps = [ps_o.tile([HD + 1, QCW], F32, tag=f"ot{h2}", name=f"ot{qc}{j}{h2}")
                             for h2 in range(2)]
                    pts = {}
                    for kt in range(QT):
                        # both head-halves' scores in ONE psum tile -> ONE
                        # [128,1024] exp per kt (halves ACT instruction and
                        # semaphore overhead; attention is ACT-bound at full
                        # PE speed)
                        sps = ps_s.tile([128, 2 * QCW], F32, tag="s",
                                        name=f"sps{kt % 2}")
                        for h2 in range(2):  # h2=0 -> rows 0:64, h2=1 -> 64:128
                            nc.tensor.matmul(
                                sps[:, h2 * QCW:(h2 + 1) * QCW],
                                kt_sb[j][h2 * 64:h2 * 64 + 64,
                                         kt * 128:(kt + 1) * 128],
                                qt_sb[j][h2 * 64:h2 * 64 + 64,
                                         qc * QCW:(qc + 1) * QCW],
                                start=True, stop=True,
                                tile_position=(h2 * 64, 0),
                            )
                        pt = ptp.tile([128, 2 * QCW], ADT, tag="pt")
                        nc.scalar.activation(
                            pt[:], sps[:],
                            mybir.ActivationFunctionType.Exp,
                            scale=1.0 / SCALE,
                        )
                        for h2 in range(2):
                            h = 2 * j + h2
                            nc.tensor.matmul(
                                ot_ps[h2][:],
                                v_sb[kt][:, h * 65:h * 65 + 65],
                                pt[:, h2 * QCW:(h2 + 1) * QCW],
                                start=(kt == 0), stop=(kt == QT - 1),
                            )
                    # evacuate O.T+sums to SBUF fast (frees the psum slot
                    # for the next head pair), then normalize off-path
                    for h2 in range(2):
                        h = 2 * j + h2
                        otr = nrm.tile([HD + 1, QCW], BF16, tag="otr")
                        nc.vector.tensor_copy(otr[:], ot_ps[h2][:])
                        rc32 = nrm.tile([1, QCW], BF16, tag="rc32")
                        with nc.allow_low_precision(reason="bf16 softmax denom; tol 2e-2"):
                            nc.vector.reciprocal(rc32[:], otr[HD:HD + 1, :])
                        if qc == QCN - 1 and j == 1:
                            # tail-critical normalize: broadcast 1/rowsum via a
                            # one-row PE matmul (PE is idle here; gpsimd's
                            # software broadcast would sit on the outproj path)
                            sc_ps = ps_s.tile([128, 2 * QCW], F32, tag="s",
                                              name=f"scps{h2}")
                            nc.tensor.matmul(sc_ps[0:HD, 0:QCW], ones_row[:],
                                             rc32[:], start=True, stop=True)
                            nc.vector.tensor_mul(
                                otn_sb[j][h2 * HD:(h2 + 1) * HD,
                                          qc * QCW:(qc + 1) * QCW],
                                otr[0:HD, :], sc_ps[0:HD, 0:QCW],
                            )
                        else:
                            sc = nrm.tile([HD, QCW], BF16, tag="sc")
                            nc.gpsimd.partition_broadcast(sc[:], rc32[:])
                            nc.vector.tensor_mul(
                                otn_sb[j][h2 * HD:(h2 + 1) * HD,
                                          qc * QCW:(qc + 1) * QCW],
                                otr[0:HD, :], sc[:],
                            )
                pending.append(qc)
            emit_outproj(pending.pop())

    nc.compile()
    return nc


_NC_CACHE = None


def _get_nc():
    global _NC_CACHE
    if _NC_CACHE is None:
        _NC_CACHE = build_nc()
    return _NC_CACHE


def _pack_w(Whs):
    """W[256, 1024] -> W.T [1024, 256] -> [128, 8*256] with row p holding
    contraction rows {p, 128+p, ...}: one contiguous 4KB DMA line per
    partition."""
    import ml_dtypes
    wt = np.ascontiguousarray(np.asarray(Whs, np.float32).T)  # [1024, 256]
    packed = wt.reshape(8, 128, 256).transpose(1, 0, 2).reshape(128, 8 * 256)
    return np.ascontiguousarray(packed).astype(ml_dtypes.bfloat16)


def shard_inputs(query, key, value, Wq, bq, Wk, bk, Wv, bv, Wo, bo):
    """Build the 8 per-core input maps (host-side shard + transpose)."""
    import ml_dtypes
    f = np.float32
    bf = ml_dtypes.bfloat16
    in_maps = []
    for c in range(NCORES):
        b = c // 4
        g = c % 4
        hs = slice(g * DPC, (g + 1) * DPC)
        in_maps.append({
            "xq_t": np.ascontiguousarray(np.asarray(query[b], f).T).astype(bf),
            "xk_t": np.ascontiguousarray(np.asarray(key[b], f).T).astype(bf),
            "xv_t": np.ascontiguousarray(np.asarray(value[b], f).T).astype(bf),
            "wq_t": _pack_w(Wq[hs, :]),
            "wk_t": _pack_w(Wk[hs, :]),
            "wv_t": _pack_w(Wv[hs, :]),
            "wo_t": np.ascontiguousarray(np.asarray(Wo[:, hs], f).T).astype(bf),
            "ball": np.stack([np.asarray(b[hs], f).reshape(2, 128)[hp]
                              for b in (bq, bk, bv) for hp in range(2)],
                             axis=1).copy(),
            "ident": np.eye(128, dtype=f).astype(bf),
        })
    return in_maps


def kernel(query, key, value, Wq, bq, Wk, bk, Wv, bv, Wo, bo, **run_kwargs):
    nc = _get_nc()
    in_maps = shard_inputs(query, key, value, Wq, bq, Wk, bk, Wv, bv, Wo, bo)
    res = run_bass_kernel_spmd(nc, in_maps, core_ids=list(range(NCORES)),
                               **run_kwargs)
    out = np.zeros((B, S, D), np.float32)
    for c in range(NCORES):
        out[c // 4] += np.asarray(res.results[c]["y"], np.float32)
    out += np.asarray(bo, np.float32)
    if run_kwargs:
        kernel.last_result = res
    return out



# revision 40
# speedup vs baseline: 1.0152x; 1.0063x over previous
"""Multi-head attention (B=2, S=2048, D=1024, H=16) on 8 NeuronCores.

Sharding: core c -> batch c//4, head-group c%4 (4 heads, 256 proj dims).
Per-core Bass/Tile kernel: bf16 Q/K/V projections (weight-stationary),
transposed-scores attention in 512-wide q-chunks (k on partitions,
softmax without max-subtraction via a ones-column rowsum), row-parallel
partial out-projection. Host sums the 4 bf16 partials per batch + bo.

Perf notes (HW-measured):
- All matmul operands bf16 (f32r moving data doubles SBUF read power and
  feeds the chip's activity throttle; fp8 DoubleRow would halve PE rows
  but its ~5% rms error fails the 2e-2 gate).
- TRN2 duty-cycle controller: sustained PE+ACT concurrency escalates to
  a 50% utilization clamp after ~45us; low-activity phases (projections,
  out-proj) run unthrottled. 512-wide chunks + out-proj interleaved at
  the second head-pair keep activity dips frequent so the controller
  keeps de-escalating.
- Weights/biases host-packed so every SBUF partition row is one
  contiguous DRAM line (one DMA per tensor, 4KB packets instead of
  128x512B).
- Out-proj PSUM tiles staged to SBUF via ACT-engine Copy (DVE is busier;
  GpSimd cannot read PSUM); y shipped bf16.
- exp on ACT is the irreducible floor (~131k lanes-cycles/core); scores
  psum double-buffered (bufs=2) so scores run ahead of exp.
- Non-critical const DMAs (wq/wk/wo/biases/ident) are dribbled one per
  x-chunk behind the first V-proj loads: the Sync queue issues in order
  at ~0.7us per DMA, so front-loading them delays the first matmul.
- DVE reciprocal() is a ~3.3us multipass op per [1,512] call; the
  tail-critical last-chunk normalize broadcasts 1/rowsum via a one-row
  PE matmul instead of gpsimd (PE is idle there). reciprocal_approx_*
  custom-DVE ops produce garbage on this hardware; ACT Reciprocal is
  hard-blocked by bass.
- Both head-halves' scores share one [128,1024] psum tile so each kt
  step needs ONE exp instead of two: halving ACT instruction/semaphore
  rate dropped the chip's activity escalation below threshold
  (throttle_active 110us -> 28us) and made the attention phase purely
  ACT-exp-bound at ~1.04us/kt.
- Q-projection chunks 1-3 are deferred into the attention phase as
  pair-boundary PE filler (their x-chunks stay in the xin rotation),
  starting the exp stream ~14us earlier. The same trick for K regressed
  (mid-kt-loop filler bubbles the saturated ACT stream) - pair
  boundaries are the only safe filler slots.
"""

import sys

sys.path.insert(0, "/opt/trn_rl_repo")

from contextlib import ExitStack

import numpy as np

import concourse.bacc as bacc
import concourse.mybir as mybir
import concourse.tile as tile
from concourse.bass_utils import run_bass_kernel_spmd

B = 2
S = 2048
D = 1024
H = 16
HD = 64
HPC = 4          # heads per core
DPC = HPC * HD   # 256 projection dims per core
NCORES = 8
SCALE = 8.0      # sqrt(HD)

F32 = mybir.dt.float32
F32R = mybir.dt.float32r
BF16 = mybir.dt.bfloat16
ADT = BF16   # attention operand dtype (qt/kt/v/pt)

DCH = D // 128   # 8 contraction chunks of 128
QT = S // 128    # 16 q-tiles / k-tiles of 128
QCN = 4          # attention q-chunks of 512
QCW = 512


def build_nc():
    nc = bacc.Bacc("TRN2", target_bir_lowering=False, debug=False, num_devices=NCORES)

    xq = nc.dram_tensor("xq_t", [D, S], BF16, kind="ExternalInput")
    xk = nc.dram_tensor("xk_t", [D, S], BF16, kind="ExternalInput")
    xv = nc.dram_tensor("xv_t", [D, S], BF16, kind="ExternalInput")
    wq = nc.dram_tensor("wq_t", [128, D // 128 * DPC], BF16, kind="ExternalInput")
    wk = nc.dram_tensor("wk_t", [128, D // 128 * DPC], BF16, kind="ExternalInput")
    wv = nc.dram_tensor("wv_t", [128, D // 128 * DPC], BF16, kind="ExternalInput")
    wo = nc.dram_tensor("wo_t", [DPC, D], BF16, kind="ExternalInput")
    ball = nc.dram_tensor("ball", [128, 6], F32, kind="ExternalInput")
    ident = nc.dram_tensor("ident", [128, 128], BF16, kind="ExternalInput")
    y = nc.dram_tensor("y", [S, D], BF16, kind="ExternalOutput")

    with tile.TileContext(nc) as tc, ExitStack() as ctx:
        const = ctx.enter_context(tc.tile_pool(name="const", bufs=1))
        xin = ctx.enter_context(tc.tile_pool(name="xin", bufs=8))
        qkv = ctx.enter_context(tc.tile_pool(name="qkv", bufs=1))
        yp = ctx.enter_context(tc.tile_pool(name="yp", bufs=3))
        ptp = ctx.enter_context(tc.tile_pool(name="ptp", bufs=3))
        nrm = ctx.enter_context(tc.tile_pool(name="nrm", bufs=2))
        # tag semantics: same tag -> rotate through `bufs` slots;
        # distinct tags -> independent allocations.

        # ---- constants / weights ----
        # tiny dummy exp first: preloads the ACT exp table set off the
        # critical path (a ~4us PE-idle gap at attention start re-throttles
        # the PE clock to 1.2GHz for the rest of the kernel otherwise)
        dmy = const.tile([1, 16], F32, tag="dmy")
        nc.vector.memset(dmy[:], 0.0)
        dmy2 = const.tile([1, 16], F32, tag="dmy2")
        nc.scalar.activation(dmy2[:], dmy[:], mybir.ActivationFunctionType.Exp)

        # memset can't target f32r; stage in f32 and round via DVE copy
        ones32 = const.tile([1, 128], F32, tag="ones32")
        nc.vector.memset(ones32[:], 1.0)
        ones = const.tile([1, 128], F32R, tag="ones")
        nc.vector.tensor_copy(ones[:], ones32[:])
        ones_r32 = const.tile([1, HD], F32, tag="ones_r32")
        nc.vector.memset(ones_r32[:], 1.0)
        ones_row = const.tile([1, HD], BF16, tag="ones_row")
        nc.vector.tensor_copy(ones_row[:], ones_r32[:])
        onesv32 = const.tile([128, HPC], F32, tag="onesv32")
        nc.vector.memset(onesv32[:], 1.0)
        onesv = const.tile([128, HPC], F32R, tag="onesv")
        nc.vector.tensor_copy(onesv[:], onesv32[:])
        # packed weights: one contiguous-row DMA per tensor (vs 8x128
        # 512B-packet transfers); w*_sb[d] are views into the packed tile
        wq_all = const.tile([128, DCH * DPC], BF16, tag="wqa", name="wqa")
        wk_all = const.tile([128, DCH * DPC], BF16, tag="wka", name="wka")
        wv_all = const.tile([128, DCH * DPC], BF16, tag="wva", name="wva")
        nc.sync.dma_start(wv_all[:, 0:DPC], wv[:, 0:DPC])
        nc.sync.dma_start(wv_all[:, DPC:], wv[:, DPC:])
        ball_sb = const.tile([128, 6], F32, tag="ball", name="ball")
        # deferred until after the first x-chunk DMA is issued: only wv and
        # xt(v,0) gate the first matmul, and the Sync queue issues in order
        deferred_dmas = [
            lambda: nc.sync.dma_start(wq_all[:], wq[:]),
            lambda: nc.sync.dma_start(wk_all[:], wk[:]),
            lambda: nc.sync.dma_start(ball_sb[:], ball[:]),
        ]
        wq_sb = [wq_all[:, d * DPC:(d + 1) * DPC] for d in range(DCH)]
        wk_sb = [wk_all[:, d * DPC:(d + 1) * DPC] for d in range(DCH)]
        wv_sb = [wv_all[:, d * DPC:(d + 1) * DPC] for d in range(DCH)]
        wo_sb = [const.tile([128, D], BF16, tag=f"wo{g}", name=f"wo{g}") for g in range(2)]
        for g in range(2):
            deferred_dmas.append(
                lambda g=g: nc.sync.dma_start(wo_sb[g][:], wo[g * 128:(g + 1) * 128, :]))
        bq_sb = [ball_sb[:, hp:hp + 1] for hp in range(2)]
        bk_sb = [ball_sb[:, 2 + hp:3 + hp] for hp in range(2)]
        bv_sb = [ball_sb[:, 4 + hp:5 + hp] for hp in range(2)]
        id_sb = const.tile([128, 128], BF16, tag="id")
        deferred_dmas.append(lambda: nc.sync.dma_start(id_sb[:], ident[:]))

        # ---- V tiles (128, 4*65) with ones column, filled by PE transpose
        # of a V.T projection (weight-stationary like Q/K; avoids the
        # per-matmul LDWEIGHTS serialization of an x-stationary V-proj) ----
        v_sb = [qkv.tile([128, HPC * (HD + 1)], ADT, tag=f"v{st}", name=f"v{st}") for st in range(QT)]
        for st in range(QT):
            v4 = v_sb[st][:].rearrange("p (h w) -> p h w", h=HPC)
            nc.vector.tensor_copy(
                v4[:, :, HD:HD + 1],
                onesv[:].rearrange("p (a b) -> p a b", b=1),
            )
        vt_sb = [qkv.tile([128, S], BF16, tag=f"vt{hp}", name=f"vtt{hp}") for hp in range(2)]

        # ---- Q.T / K.T projections: (d'=hp*128 partitions, s free) ----
        qt_sb = [qkv.tile([128, S], ADT, tag=f"qt{hp}", name=f"qtt{hp}") for hp in range(2)]
        kt_sb = [qkv.tile([128, S], ADT, tag=f"kt{hp}", name=f"ktt{hp}") for hp in range(2)]
        with tc.tile_pool(name="ps_p", bufs=1, space="PSUM") as ps_p:
            xq_tiles = []
            for which, xin_dram, w_sb, b_sb, dst in (
                ("v", xv, wv_sb, bv_sb, vt_sb),
                ("k", xk, wk_sb, bk_sb, kt_sb),
                ("q", xq, wq_sb, bq_sb, qt_sb),
            ):
                pcs = [0] if which == "q" else [0, 1, 2, 3]
                accs = {}
                for hp in range(2):
                    for pc in pcs:
                        accs[(hp, pc)] = ps_p.tile([128, 512], F32, tag=f"pp{hp * 4 + pc}", name=f"pp_{which}{hp}{pc}")
                for d in range(DCH):
                    xt = xin.tile([128, S], BF16, tag="x")
                    if which == "v" and d == 0:
                        nc.sync.dma_start(xt[:, 0:512],
                                          xin_dram[0:128, 0:512])
                        nc.sync.dma_start(xt[:, 512:],
                                          xin_dram[0:128, 512:])
                    else:
                        nc.sync.dma_start(xt[:], xin_dram[d * 128:(d + 1) * 128, :])
                    if which == "q":
                        xq_tiles.append(xt)
                    if deferred_dmas:
                        deferred_dmas.pop(0)()
                    for hp in range(2):
                        for pc in pcs:
                            nc.tensor.matmul(
                                accs[(hp, pc)][:],
                                w_sb[d][:, hp * 128:(hp + 1) * 128],
                                xt[:, pc * 512:(pc + 1) * 512],
                                start=(d == 0), stop=(d == DCH - 1),
                            )
                for hp in range(2):
                    for pc in pcs:
                        nc.vector.tensor_scalar_add(
                            dst[hp][:, pc * 512:(pc + 1) * 512],
                            accs[(hp, pc)][:],
                            b_sb[hp][:],
                        )
            # V.T -> V transposes last: dense PE work (~8us) bridging the
            # proj->attention boundary while K/Q evacuations drain, so the
            # PE clock stays un-throttled into the attention phase
            for hp in range(2):
                for st in range(QT):
                    tp = ps_p.tile([128, 128], BF16, tag=f"pp{st % 8}",
                                   name=f"tp{hp}{st}")
                    nc.tensor.transpose(
                        tp[:],
                        vt_sb[hp][:, st * 128:(st + 1) * 128],
                        id_sb[:],
                    )
                    v4 = v_sb[st][:].rearrange("p (h w) -> p h w", h=HPC)
                    nc.vector.tensor_copy(
                        v4[:, 2 * hp:2 * hp + 2, 0:HD],
                        tp[:].rearrange("p (h w) -> p h w", h=2),
                    )

        # ---- attention + normalization, head-pairs packed on PE rows ----
        otn_sb = [qkv.tile([128, S], BF16, tag=f"otn{j}", name=f"otn{j}") for j in range(2)]
        with tc.tile_pool(name="ps_s", bufs=2, space="PSUM") as ps_s, \
             tc.tile_pool(name="ps_o", bufs=1, space="PSUM") as ps_o, \
             tc.tile_pool(name="ps_q", bufs=1, space="PSUM") as ps_q:

            def emit_qproj(pc):
                # deferred Q-projection chunk: PE filler at a pair boundary,
                # so the exp stream starts ~3 chunks of q-proj earlier
                for hp in range(2):
                    acc = ps_q.tile([128, 512], F32, tag=f"qp{hp}",
                                    name=f"qp{pc}{hp}")
                    for d in range(DCH):
                        nc.tensor.matmul(
                            acc[:],
                            wq_sb[d][:, hp * 128:(hp + 1) * 128],
                            xq_tiles[d][:, pc * 512:(pc + 1) * 512],
                            start=(d == 0), stop=(d == DCH - 1),
                        )
                    nc.vector.tensor_scalar_add(
                        qt_sb[hp][:, pc * 512:(pc + 1) * 512],
                        acc[:], bq_sb[hp][:],
                    )

            def emit_outproj(qc):
                # out-proj for a finished q-chunk; emitted during the NEXT
                # chunk's attention so its matmuls fill PE slack there
                for qt_i in range(qc * QCW // 128, (qc + 1) * QCW // 128):
                    ysb = yp.tile([128, D], BF16, tag="y", name=f"ysb{qt_i}")
                    for dc in range(2):
                        yps = ps_o.tile([128, 512], F32, tag=f"ot{dc}",
                                        name=f"yps{qt_i}{dc}")
                        for g in range(2):
                            nc.tensor.matmul(
                                yps[:],
                                otn_sb[g][:, qt_i * 128:(qt_i + 1) * 128],
                                wo_sb[g][:, dc * 512:(dc + 1) * 512],
                                start=(g == 0), stop=(g == 1),
                            )
                        nc.scalar.activation(
                            ysb[:, dc * 512:(dc + 1) * 512], yps[:],
                            mybir.ActivationFunctionType.Copy,
                        )
                    nc.sync.dma_start(y[qt_i * 128:(qt_i + 1) * 128, :], ysb[:])

            pending = []
            for qc in range(QCN):
                for j in range(2):          # head pair: heads 2j, 2j+1
                    if j == 1 and qc < 3:
                        emit_qproj(qc + 1)
                    if j == 1 and pending:
                        emit_outproj(pending.pop())
                    ot_ps = [ps_o.tile([HD + 1, QCW], F32, tag=f"ot{h2}", name=f"ot{qc}{j}{h2}")
                             for h2 in range(2)]
                    pts = {}
                    for kt in range(QT):
                        # both head-halves' scores in ONE psum tile -> ONE
                        # [128,1024] exp per kt (halves ACT instruction and
                        # semaphore overhead; attention is ACT-bound at full
                        # PE speed)
                        sps = ps_s.tile([128, 2 * QCW], F32, tag="s",
                                        name=f"sps{kt % 2}")
                        for h2 in range(2):  # h2=0 -> rows 0:64, h2=1 -> 64:128
                            nc.tensor.matmul(
                                sps[:, h2 * QCW:(h2 + 1) * QCW],
                                kt_sb[j][h2 * 64:h2 * 64 + 64,
                                         kt * 128:(kt + 1) * 128],
                                qt_sb[j][h2 * 64:h2 * 64 + 64,
                                         qc * QCW:(qc + 1) * QCW],
                                start=True, stop=True,
                                tile_position=(h2 * 64, 0),
                            )
                        pt = ptp.tile([128, 2 * QCW], ADT, tag="pt")
                        nc.scalar.activation(
                            pt[:], sps[:],
                            mybir.ActivationFunctionType.Exp,
                            scale=1.0 / SCALE,
                        )
                        for h2 in range(2):
                            h = 2 * j + h2
                            nc.tensor.matmul(
                                ot_ps[h2][:],
                                v_sb[kt][:, h * 65:h * 65 + 65],
                                pt[:, h2 * QCW:(h2 + 1) * QCW],
                                start=(kt == 0), stop=(kt == QT - 1),
                            )
                    # evacuate O.T+sums to SBUF fast (frees the psum slot
                    # for the next head pair), then normalize off-path
                    for h2 in range(2):
                        h = 2 * j + h2
                        otr = nrm.tile([HD + 1, QCW], BF16, tag="otr")
                        nc.vector.tensor_copy(otr[:], ot_ps[h2][:])
                        # partition-0-aligned fp32 staging for the fast custom
                        # reciprocal (5x cheaper than the 3.3us DVE multipass;
                        # needs fp32 and aligned base partitions)
                        sum32 = nrm.tile([1, QCW], F32, tag="sum32")
                        nc.vector.tensor_copy(sum32[:], ot_ps[h2][HD:HD + 1, :])
                        rc32f = nrm.tile([1, QCW], F32, tag="rc32f")
                        nc.vector.reciprocal_approx_fast(rc32f[:], sum32[:])
                        rc32 = nrm.tile([1, QCW], BF16, tag="rc32")
                        nc.vector.tensor_copy(rc32[:], rc32f[:])
                        if qc == QCN - 1 and j == 1:
                            # tail-critical normalize: broadcast 1/rowsum via a
                            # one-row PE matmul (PE is idle here; gpsimd's
                            # software broadcast would sit on the outproj path)
                            sc_ps = ps_s.tile([128, 2 * QCW], F32, tag="s",
                                              name=f"scps{h2}")
                            nc.tensor.matmul(sc_ps[0:HD, 0:QCW], ones_row[:],
                                             rc32[:], start=True, stop=True)
                            nc.vector.tensor_mul(
                                otn_sb[j][h2 * HD:(h2 + 1) * HD,
                                          qc * QCW:(qc + 1) * QCW],
                                otr[0:HD, :], sc_ps[0:HD, 0:QCW],
                            )
                        else:
                            sc = nrm.tile([HD, QCW], BF16, tag="sc")
                            nc.gpsimd.partition_broadcast(sc[:], rc32[:])
                            nc.vector.tensor_mul(
                                otn_sb[j][h2 * HD:(h2 + 1) * HD,
                                          qc * QCW:(qc + 1) * QCW],
                                otr[0:HD, :], sc[:],
                            )
                pending.append(qc)
            emit_outproj(pending.pop())

    nc.compile()
    return nc


_NC_CACHE = None


def _get_nc():
    global _NC_CACHE
    if _NC_CACHE is None:
        _NC_CACHE = build_nc()
    return _NC_CACHE


def _pack_w(Whs):
    """W[256, 1024] -> W.T [1024, 256] -> [128, 8*256] with row p holding
    contraction rows {p, 128+p, ...}: one contiguous 4KB DMA line per
    partition."""
    import ml_dtypes
    wt = np.ascontiguousarray(np.asarray(Whs, np.float32).T)  # [1024, 256]
    packed = wt.reshape(8, 128, 256).transpose(1, 0, 2).reshape(128, 8 * 256)
    return np.ascontiguousarray(packed).astype(ml_dtypes.bfloat16)


def shard_inputs(query, key, value, Wq, bq, Wk, bk, Wv, bv, Wo, bo):
    """Build the 8 per-core input maps (host-side shard + transpose)."""
    import ml_dtypes
    f = np.float32
    bf = ml_dtypes.bfloat16
    in_maps = []
    for c in range(NCORES):
        b = c // 4
        g = c % 4
        hs = slice(g * DPC, (g + 1) * DPC)
        in_maps.append({
            "xq_t": np.ascontiguousarray(np.asarray(query[b], f).T).astype(bf),
            "xk_t": np.ascontiguousarray(np.asarray(key[b], f).T).astype(bf),
            "xv_t": np.ascontiguousarray(np.asarray(value[b], f).T).astype(bf),
            "wq_t": _pack_w(Wq[hs, :]),
            "wk_t": _pack_w(Wk[hs, :]),
            "wv_t": _pack_w(Wv[hs, :]),
            "wo_t": np.ascontiguousarray(np.asarray(Wo[:, hs], f).T).astype(bf),
            "ball": np.stack([np.asarray(b[hs], f).reshape(2, 128)[hp]
                              for b in (bq, bk, bv) for hp in range(2)],
                             axis=1).copy(),
            "ident": np.eye(128, dtype=f).astype(bf),
        })
    return in_maps


def kernel(query, key, value, Wq, bq, Wk, bk, Wv, bv, Wo, bo, **run_kwargs):
    nc = _get_nc()
    in_maps = shard_inputs(query, key, value, Wq, bq, Wk, bk, Wv, bv, Wo, bo)
    res = run_bass_kernel_spmd(nc, in_maps, core_ids=list(range(NCORES)),
                               **run_kwargs)
    out = np.zeros((B, S, D), np.float32)
    for c in range(NCORES):
        out[c // 4] += np.asarray(res.results[c]["y"], np.float32)
    out += np.asarray(bo, np.float32)
    if run_kwargs:
        kernel.last_result = res
    return out

